# revision 32
# baseline (speedup 1.0000x reference)
"""Trainium2 Bass kernel for nn_EntropyConv (masked 5x5 PixelCNN-style conv,
per-latitude-partition padding + width masking + PReLU).

Strategy: data-parallel over batch (8 cores x 1 batch element). Per core,
a row-phase-split SBUF layout puts (row mod 4, ci) on the 128 K-partitions
so each PSUM tile computes 4 output rows x 32 channels with K=M=128 fp32r
matmuls: 10 matmuls per tile (5 kw shifts x 2 row-windows), kh taps encoded
in host-precomputed block-Toeplitz weight matrices.
"""

import sys
import os
from contextlib import ExitStack

import numpy as np

sys.path.insert(0, "/opt/trn_rl_repo")

import concourse.bass as bass  # noqa: E402
import concourse.tile as tile  # noqa: E402
from concourse import bacc, mybir  # noqa: E402
from concourse import bass_utils  # noqa: E402
from concourse.bass_utils import run_bass_kernel_spmd  # noqa: E402

# Enable walrus's redundant-LDWEIGHTS elimination: our matmul stream reuses
# each stationary weight across 8 consecutive matmuls, and the default
# --enable-ldw-opt=false forces a ~190ns weight reload per matmul (~40% of
# PE time). Correctness is verified against the fp32 reference.
if not os.environ.get("BASS_NO_LDWOPT"):
    _orig_run_command = bass_utils.run_command

    def _run_command_ldwopt(argv, **kwargs):
        argv = ["--enable-ldw-opt=true" if a == "--enable-ldw-opt=false" else a
                for a in argv]
        return _orig_run_command(argv, **kwargs)

    if bass_utils.run_command is not _run_command_ldwopt:
        bass_utils.run_command = _run_command_ldwopt

# Model constants (hardcoded per problem spec)
NGROUPS, CIN, COUT, KSIZE, NPART = 8, 4, 4, 5, 8
B, H, W = 8, 256, 512
CI = NGROUPS * CIN   # 32
CO = NGROUPS * COUT  # 32
Hp = H // NPART      # 32 rows per latitude chunk
NBLK = Hp // 4       # 8 four-row blocks per chunk
NCORES = 8
F32 = mybir.dt.float32
F32R = mybir.dt.float32r

# X4 tile: 2 guard cols + 9 blocks of 512 + 2 guard cols
XBLK = 9
XLEN = 2 + XBLK * W + 2

LAST_RESULT = None  # BassKernelResults from the most recent run (for test.py)


def _group_mask():
    """PixelCNN group mask for 5x5 kernel, mask-B (hidden) variant."""
    m = np.zeros((CO, CI, KSIZE, KSIZE), np.float32)
    c = KSIZE // 2
    m[:, :, :c, :] = 1.0
    m[:, :, c, :c] = 1.0
    gin = np.arange(CI) // CIN
    gout = np.arange(CO) // COUT
    center = gin[None, :] <= gout[:, None]
    m[:, :, c, c] = center.astype(np.float32)
    return m


def _build_weights(weight):
    """Block-Toeplitz lhsT matrices.

    w1/w2[kw, 32*rp+ci, 32*j+co]: contribution of input row (4h4+rp-2)
    [w1] or (4h4+rp+2) [w2] to output row (4h4+j), i.e. kh = rp-j [w1]
    or rp-j+4 [w2], valid when 0 <= kh < 5.
    """
    wm = (weight * _group_mask()).astype(np.float32)  # [co, ci, kh, kw]
    w1 = np.zeros((KSIZE, 128, 128), np.float32)
    w2 = np.zeros((KSIZE, 128, 128), np.float32)
    for rp in range(4):
        for j in range(4):
            kh1 = rp - j
            kh2 = rp - j + 4
            # [ci, co] block at rows 32*rp+ci, cols 4*co+j (co-major
            # output partitions -> contiguous 4-row HBM stores)
            if 0 <= kh1 < KSIZE:
                for kw in range(KSIZE):
                    w1[kw, 32 * rp:32 * rp + 32, j::4] = wm[:, :, kh1, kw].T
            if 0 <= kh2 < KSIZE:
                for kw in range(KSIZE):
                    w2[kw, 32 * rp:32 * rp + 32, j::4] = wm[:, :, kh2, kw].T
    return w1, w2


def _tile_groups(width):
    """Split the 8 output blocks of a chunk into PSUM tile groups of k
    blocks, keeping k*width <= 512 (one PSUM bank) and preferring
    k*width >= 256 (fp32r full-rate threshold)."""
    if width >= 256:
        return [(b, 1) for b in range(NBLK)]
    k = min(NBLK, 512 // width)
    groups = []
    b = 0
    while b < NBLK:
        kk = min(k, NBLK - b)
        groups.append((b, kk))
        b += kk
    return groups


def _build_program(widths, has_bias, use_prelu=True):
    nc = bacc.Bacc("TRN2", target_bir_lowering=False, debug=False,
                   num_devices=NCORES)

    x_d = nc.dram_tensor("x", [CI, H, W], F32R, kind="ExternalInput")
    w1_d = nc.dram_tensor("w1", [KSIZE, 128, 128], F32R, kind="ExternalInput")
    w2_d = nc.dram_tensor("w2", [KSIZE, 128, 128], F32R, kind="ExternalInput")
    alpha_d = nc.dram_tensor("alpha_p", [128, 1], F32, kind="ExternalInput")
    if has_bias:
        bias_d = nc.dram_tensor("bias_p", [128, 1], F32, kind="ExternalInput")
        abias_d = nc.dram_tensor("abias_p", [128, 1], F32,
                                 kind="ExternalInput")
    y_d = nc.dram_tensor("y", [CO, H, W], F32, kind="ExternalOutput")

    # DRAM views for phase-split access
    # x rows: global row = 4*hb + r
    x_r = x_d.ap().rearrange("ci (hb r) w -> r ci hb w", r=4)

    with tile.TileContext(nc) as tc, ExitStack() as ctx:
        wpool = ctx.enter_context(tc.tile_pool(name="wts", bufs=1))
        spool = ctx.enter_context(tc.tile_pool(name="scalars", bufs=1))
        x4pool = ctx.enter_context(tc.tile_pool(name="x4", bufs=3))
        psumpool = ctx.enter_context(
            tc.tile_pool(name="psum", bufs=8, space=bass.MemorySpace.PSUM))
        outpool = ctx.enter_context(tc.tile_pool(name="outsb", bufs=24))
        azpool = ctx.enter_context(tc.tile_pool(name="azp", bufs=6))

        wt1 = wpool.tile([128, KSIZE, 128], F32R, tag="w1")
        wt2 = wpool.tile([128, KSIZE, 128], F32R, tag="w2")
        w1v = w1_d.ap().rearrange("kw k m -> k kw m")
        w2v = w2_d.ap().rearrange("kw k m -> k kw m")
        for kw in range(KSIZE):
            nc.sync.dma_start(wt1[:, kw, :], w1v[:, kw, :])
            nc.scalar.dma_start(wt2[:, kw, :], w2v[:, kw, :])
        alpha_t = spool.tile([128, 1], F32, tag="alpha")
        nc.sync.dma_start(alpha_t[:], alpha_d.ap())
        if has_bias:
            bias_t = spool.tile([128, 1], F32, tag="bias")
            nc.sync.dma_start(bias_t[:], bias_d.ap())
            abias_t = spool.tile([128, 1], F32, tag="abias")
            nc.sync.dma_start(abias_t[:], abias_d.ap())

        prev_mm = [None]

        # PE warmup: tiny scratch matmuls fill the HAM activity window
        # during the first x4 load, so real matmuls start at 2.4 GHz
        warm = wpool.tile([128, 128], F32R, tag="warm")
        warm_ps = psumpool.tile([128, 16], F32, tag="ps")
        nc.vector.memset(warm[:, :].bitcast(F32), 0.0)
        for _ in range(56):
            mm = nc.tensor.matmul(warm_ps[:, :], warm[:, :], warm[:, 0:16],
                                  start=True, stop=True)
            if prev_mm[0] is not None:
                bass._add_dep_helper(mm.ins, prev_mm[0].ins, sync=False,
                                     reason="pe-stream-order")
            prev_mm[0] = mm

        store_eng = [0]
        for p in range(NPART):
            width = widths[p]
            x4 = x4pool.tile([128, XLEN], F32R, tag="x4")
            x4f = x4[:, :]

            # load the 4 rp groups (valid cols only)
            for rp in range(4):
                if rp < 2:
                    r, bdst = rp + 2, 1
                    # rows 4b+rp-2 for b=1..8 -> r=(rp+2), hb = p*8 + b-1
                    src = x_r[r][:, p * NBLK:p * NBLK + NBLK, 0:width]
                else:
                    r = rp - 2
                    # rows 4b+rp-2 for b=0..7 -> r=(rp-2), hb = p*8 + b
                    src = x_r[r][:, p * NBLK:p * NBLK + NBLK, 0:width]
                    bdst = 0
                dst = x4f[32 * rp:32 * rp + 32,
                          2 + bdst * W:2 + (bdst + NBLK) * W].rearrange(
                              "q (b x) -> q b x", x=W)[:, :, 0:width]
                (nc.gpsimd if rp < 2 else nc.sync).dma_start(dst, src)

            # guards (left/right 2 cols)
            nc.vector.memset(x4f[:, 0:2].bitcast(F32), 0.0)
            nc.vector.memset(x4f[:, XLEN - 2:XLEN].bitcast(F32), 0.0)
            # pad blocks: rp 0,1 -> block 0 ; rp 2,3 -> block 8
            nc.vector.memset(x4f[0:64, 2:2 + W].bitcast(F32), 0.0)
            nc.vector.memset(x4f[64:128, 2 + 8 * W:2 + 9 * W].bitcast(F32), 0.0)
            # 2-col zero strips: [width, width+2) and [510, 512) in each block
            blocks_view = x4f[:, 2:2 + XBLK * W].rearrange(
                "q (b x) -> q b x", x=W)
            nc.vector.memset(blocks_view[:, :, width:width + 2].bitcast(F32), 0.0)
            if width + 2 < W - 2:
                nc.vector.memset(blocks_view[:, :, W - 2:W].bitcast(F32), 0.0)

            all_groups = _tile_groups(width)
            halves = [all_groups[:(len(all_groups) + 1) // 2],
                      all_groups[(len(all_groups) + 1) // 2:]]

            for groups in halves:
              if not groups:
                  continue
              psums = []
              for (b0, k) in groups:
                ps_t = psumpool.tile([128, k * width], F32, tag="ps")
                psums.append(ps_t)

              # weight-major: each stationary weight is reused across all
              # groups back-to-back so walrus's ldw-opt elides the reloads
              NW = 2 * KSIZE
              for wi in range(NW):
                m, kw = divmod(wi, KSIZE)
                lhsT = (wt1 if m == 0 else wt2)[:, kw, :]
                for gi, (b0, k) in enumerate(groups):
                    s = 2 + (b0 + m) * W + (kw - 2)
                    rhs = x4f[:, s:s + k * W].rearrange(
                        "q (b x) -> q b x", x=W)[:, :, 0:width]
                    pview = psums[gi][:, :].rearrange(
                        "q (b x) -> q b x", x=width)
                    mm = nc.tensor.matmul(
                        pview,
                        lhsT,
                        rhs,
                        start=(wi == 0),
                        stop=(wi == NW - 1),
                    )
                    if prev_mm[0] is not None:
                        bass._add_dep_helper(
                            mm.ins, prev_mm[0].ins, sync=False,
                            reason="pe-stream-order")
                    prev_mm[0] = mm

              for gi, (b0, k) in enumerate(groups):
                n = k * width
                psum = psums[gi]
                out_sb = outpool.tile([128, n], F32, tag="osb")
                pflat = psum[:, :]
                if use_prelu:
                    # single ACT op: out = prelu(psum + bias, alpha)
                    # (HW-verified exact; CoreSim lacks Prelu)
                    nc.scalar.activation(
                        out_sb[:, :], pflat,
                        mybir.ActivationFunctionType.Prelu,
                        bias=(bias_t[:, :] if has_bias else 0.0),
                        scale=1.0, alpha=alpha_t[:, :])
                else:
                    az = azpool.tile([128, n], F32, tag="az")
                    nc.vector.tensor_copy(az[:, :], pflat)
                    nc.vector.scalar_tensor_tensor(
                        out_sb[:, :], az[:, :], alpha_t[:, :], az[:, :],
                        mybir.AluOpType.mult, mybir.AluOpType.max)

                # one store per 4-row block: y[co, 4hb:4hb+4, :width]
                # <- [128=4co+j, width]; co-major partitions match dst order
                for bb in range(k):
                    hb = p * NBLK + b0 + bb
                    dst = y_d.ap()[:, 4 * hb:4 * hb + 4, 0:width]
                    store_eng[0] += 1
                    # near the end, drain stores on two queues
                    if p >= NPART - 2 and store_eng[0] % 2 == 0:
                        eng = nc.scalar
                    else:
                        eng = nc.sync
                    eng.dma_start(dst,
                                  out_sb[:, bb * width:(bb + 1) * width])

    nc.compile()
    return nc


def kernel(x, weight, bias, alpha, widths, _trace=False):
    global LAST_RESULT
    x = np.ascontiguousarray(np.asarray(x, dtype=np.float32))
    weight = np.asarray(weight, dtype=np.float32)
    bias = np.asarray(bias, dtype=np.float32)
    alpha = np.asarray(alpha, dtype=np.float32)
    widths_np = np.asarray(widths, dtype=np.int32)
    wlist = [int(v) for v in widths_np]
    assert x.shape == (B, CI, H, W)
    for wv in wlist:
        # the block-wraparound trick requires masked-zero cols at [510,512)
        assert 4 <= wv <= W - 6, f"width {wv} outside supported range"

    w1, w2 = _build_weights(weight)
    alpha_p = np.ascontiguousarray(np.repeat(alpha, 4)[:, None].astype(np.float32))
    has_bias = bool(np.any(bias != 0.0))

    nc = _build_program(wlist, has_bias)

    shared = {"w1": w1, "w2": w2, "alpha_p": alpha_p}
    if has_bias:
        shared["bias_p"] = np.ascontiguousarray(
            np.repeat(bias, 4)[:, None].astype(np.float32))
        shared["abias_p"] = np.ascontiguousarray(
            (np.repeat(bias, 4) * np.repeat(alpha, 4))[:, None].astype(np.float32))
    in_maps = [dict(shared, x=np.ascontiguousarray(x[b])) for b in range(B)]

    res = run_bass_kernel_spmd(nc, in_maps, list(range(NCORES)),
                               trace=_trace)
    LAST_RESULT = res
    y = np.stack([res.results[c]["y"] for c in range(NCORES)], axis=0)
    return y.astype(np.float32)


if __name__ == "__main__":
    # smoke test with random data (no reference comparison)
    rng = np.random.default_rng(0)
    x = rng.standard_normal((B, CI, H, W), dtype=np.float32)
    weight = (rng.standard_normal((CO, CI, 5, 5)) * 0.05).astype(np.float32)
    bias = np.zeros(CO, np.float32)
    alpha = np.full(CO, 0.25, np.float32)
    lat = (np.arange(NPART) + 0.5) / NPART * np.pi - np.pi / 2.0
    widths = np.maximum(((np.cos(lat) * W).astype(np.int32) // 2) * 2, 16)
    y = kernel(x, weight, bias, alpha, widths.astype(np.int32))
    print("out", y.shape, y.dtype, float(np.abs(y).max()))


# revision 33
# speedup vs baseline: 1.1580x; 1.1580x over previous
"""Trainium2 Bass kernel for nn_EntropyConv (masked 5x5 PixelCNN-style conv,
per-latitude-partition padding + width masking + PReLU).

Strategy: data-parallel over batch (8 cores x 1 batch element). Per core,
a row-phase-split SBUF layout puts (row mod 4, ci) on the 128 K-partitions
so each PSUM tile computes 4 output rows x 32 channels with K=M=128 fp32r
matmuls: 10 matmuls per tile (5 kw shifts x 2 row-windows), kh taps encoded
in host-precomputed block-Toeplitz weight matrices.
"""

import sys
import os
from contextlib import ExitStack

import numpy as np

sys.path.insert(0, "/opt/trn_rl_repo")

import concourse.bass as bass  # noqa: E402
import concourse.tile as tile  # noqa: E402
from concourse import bacc, mybir  # noqa: E402
from concourse import bass_utils  # noqa: E402
from concourse.bass_utils import run_bass_kernel_spmd  # noqa: E402

# Enable walrus's redundant-LDWEIGHTS elimination: our matmul stream reuses
# each stationary weight across 8 consecutive matmuls, and the default
# --enable-ldw-opt=false forces a ~190ns weight reload per matmul (~40% of
# PE time). Correctness is verified against the fp32 reference.
if not os.environ.get("BASS_NO_LDWOPT"):
    _orig_run_command = bass_utils.run_command

    def _run_command_ldwopt(argv, **kwargs):
        argv = ["--enable-ldw-opt=true" if a == "--enable-ldw-opt=false" else a
                for a in argv]
        return _orig_run_command(argv, **kwargs)

    if bass_utils.run_command is not _run_command_ldwopt:
        bass_utils.run_command = _run_command_ldwopt

# Model constants (hardcoded per problem spec)
NGROUPS, CIN, COUT, KSIZE, NPART = 8, 4, 4, 5, 8
B, H, W = 8, 256, 512
CI = NGROUPS * CIN   # 32
CO = NGROUPS * COUT  # 32
Hp = H // NPART      # 32 rows per latitude chunk
NBLK = Hp // 4       # 8 four-row blocks per chunk
NCORES = 8
F32 = mybir.dt.float32
F32R = mybir.dt.float32r

# X4 tile: 2 guard cols + 9 blocks of 512 + 2 guard cols
XBLK = 9
XLEN = 2 + XBLK * W + 2

LAST_RESULT = None  # BassKernelResults from the most recent run (for test.py)


def _group_mask():
    """PixelCNN group mask for 5x5 kernel, mask-B (hidden) variant."""
    m = np.zeros((CO, CI, KSIZE, KSIZE), np.float32)
    c = KSIZE // 2
    m[:, :, :c, :] = 1.0
    m[:, :, c, :c] = 1.0
    gin = np.arange(CI) // CIN
    gout = np.arange(CO) // COUT
    center = gin[None, :] <= gout[:, None]
    m[:, :, c, c] = center.astype(np.float32)
    return m


def _build_weights(weight):
    """Block-Toeplitz lhsT matrices.

    w1/w2[kw, 32*rp+ci, 32*j+co]: contribution of input row (4h4+rp-2)
    [w1] or (4h4+rp+2) [w2] to output row (4h4+j), i.e. kh = rp-j [w1]
    or rp-j+4 [w2], valid when 0 <= kh < 5.
    """
    wm = (weight * _group_mask()).astype(np.float32)  # [co, ci, kh, kw]
    w1 = np.zeros((KSIZE, 128, 128), np.float32)
    w2 = np.zeros((KSIZE, 128, 128), np.float32)
    for rp in range(4):
        for j in range(4):
            kh1 = rp - j
            kh2 = rp - j + 4
            # [ci, co] block at rows 32*rp+ci, cols 4*co+j (co-major
            # output partitions -> contiguous 4-row HBM stores)
            if 0 <= kh1 < KSIZE:
                for kw in range(KSIZE):
                    w1[kw, 32 * rp:32 * rp + 32, j::4] = wm[:, :, kh1, kw].T
            if 0 <= kh2 < KSIZE:
                for kw in range(KSIZE):
                    w2[kw, 32 * rp:32 * rp + 32, j::4] = wm[:, :, kh2, kw].T
    return w1, w2


def _tile_groups(width):
    """Split the 8 output blocks of a chunk into PSUM tile groups of k
    blocks, keeping k*width <= 512 (one PSUM bank) and preferring
    k*width >= 256 (fp32r full-rate threshold)."""
    if width >= 256:
        return [(b, 1) for b in range(NBLK)]
    k = min(NBLK, 512 // width)
    groups = []
    b = 0
    while b < NBLK:
        kk = min(k, NBLK - b)
        groups.append((b, kk))
        b += kk
    return groups


def _build_program(widths, has_bias, use_prelu=True):
    nc = bacc.Bacc("TRN2", target_bir_lowering=False, debug=False,
                   num_devices=NCORES)

    x_d = nc.dram_tensor("x", [CI, H, W], F32R, kind="ExternalInput")
    w1_d = nc.dram_tensor("w1", [KSIZE, 128, 128], F32R, kind="ExternalInput")
    w2_d = nc.dram_tensor("w2", [KSIZE, 128, 128], F32R, kind="ExternalInput")
    alpha_d = nc.dram_tensor("alpha_p", [128, 1], F32, kind="ExternalInput")
    if has_bias:
        bias_d = nc.dram_tensor("bias_p", [128, 1], F32, kind="ExternalInput")
        abias_d = nc.dram_tensor("abias_p", [128, 1], F32,
                                 kind="ExternalInput")
    y_d = nc.dram_tensor("y", [CO, H, W], F32, kind="ExternalOutput")

    # DRAM views for phase-split access
    # x rows: global row = 4*hb + r
    x_r = x_d.ap().rearrange("ci (hb r) w -> r ci hb w", r=4)

    with tile.TileContext(nc) as tc, ExitStack() as ctx:
        wpool = ctx.enter_context(tc.tile_pool(name="wts", bufs=1))
        spool = ctx.enter_context(tc.tile_pool(name="scalars", bufs=1))
        x4pool = ctx.enter_context(tc.tile_pool(name="x4", bufs=3))
        psumpool = ctx.enter_context(
            tc.tile_pool(name="psum", bufs=8, space=bass.MemorySpace.PSUM))
        outpool = ctx.enter_context(tc.tile_pool(name="outsb", bufs=24))
        azpool = ctx.enter_context(tc.tile_pool(name="azp", bufs=6))

        wt1 = wpool.tile([128, KSIZE, 128], F32R, tag="w1")
        wt2 = wpool.tile([128, KSIZE, 128], F32R, tag="w2")
        w1v = w1_d.ap().rearrange("kw k m -> k kw m")
        w2v = w2_d.ap().rearrange("kw k m -> k kw m")
        for kw in range(KSIZE):
            nc.sync.dma_start(wt1[:, kw, :], w1v[:, kw, :])
            nc.scalar.dma_start(wt2[:, kw, :], w2v[:, kw, :])
        alpha_t = spool.tile([128, 1], F32, tag="alpha")
        nc.sync.dma_start(alpha_t[:], alpha_d.ap())
        if has_bias:
            bias_t = spool.tile([128, 1], F32, tag="bias")
            nc.sync.dma_start(bias_t[:], bias_d.ap())
            abias_t = spool.tile([128, 1], F32, tag="abias")
            nc.sync.dma_start(abias_t[:], abias_d.ap())

        prev_mm = [None]

        store_eng = [0]
        for p in range(NPART):
            width = widths[p]
            x4 = x4pool.tile([128, XLEN], F32R, tag="x4")
            x4f = x4[:, :]

            # load the 4 rp groups (valid cols only)
            for rp in range(4):
                if rp < 2:
                    r, bdst = rp + 2, 1
                    # rows 4b+rp-2 for b=1..8 -> r=(rp+2), hb = p*8 + b-1
                    src = x_r[r][:, p * NBLK:p * NBLK + NBLK, 0:width]
                else:
                    r = rp - 2
                    # rows 4b+rp-2 for b=0..7 -> r=(rp-2), hb = p*8 + b
                    src = x_r[r][:, p * NBLK:p * NBLK + NBLK, 0:width]
                    bdst = 0
                dst = x4f[32 * rp:32 * rp + 32,
                          2 + bdst * W:2 + (bdst + NBLK) * W].rearrange(
                              "q (b x) -> q b x", x=W)[:, :, 0:width]
                nc.gpsimd.dma_start(dst, src)

            # guards (left/right 2 cols)
            nc.vector.memset(x4f[:, 0:2].bitcast(F32), 0.0)
            nc.vector.memset(x4f[:, XLEN - 2:XLEN].bitcast(F32), 0.0)
            # pad blocks: rp 0,1 -> block 0 ; rp 2,3 -> block 8
            nc.vector.memset(x4f[0:64, 2:2 + W].bitcast(F32), 0.0)
            nc.vector.memset(x4f[64:128, 2 + 8 * W:2 + 9 * W].bitcast(F32), 0.0)
            # 2-col zero strips: [width, width+2) and [510, 512) in each block
            blocks_view = x4f[:, 2:2 + XBLK * W].rearrange(
                "q (b x) -> q b x", x=W)
            nc.vector.memset(blocks_view[:, :, width:width + 2].bitcast(F32), 0.0)
            if width + 2 < W - 2:
                nc.vector.memset(blocks_view[:, :, W - 2:W].bitcast(F32), 0.0)

            all_groups = _tile_groups(width)
            halves = [all_groups[:(len(all_groups) + 1) // 2],
                      all_groups[(len(all_groups) + 1) // 2:]]

            for groups in halves:
              if not groups:
                  continue
              psums = []
              for (b0, k) in groups:
                ps_t = psumpool.tile([128, k * width], F32, tag="ps")
                psums.append(ps_t)

              # weight-major: each stationary weight is reused across all
              # groups back-to-back so walrus's ldw-opt elides the reloads
              NW = 2 * KSIZE
              for wi in range(NW):
                m, kw = divmod(wi, KSIZE)
                lhsT = (wt1 if m == 0 else wt2)[:, kw, :]
                for gi, (b0, k) in enumerate(groups):
                    s = 2 + (b0 + m) * W + (kw - 2)
                    rhs = x4f[:, s:s + k * W].rearrange(
                        "q (b x) -> q b x", x=W)[:, :, 0:width]
                    pview = psums[gi][:, :].rearrange(
                        "q (b x) -> q b x", x=width)
                    mm = nc.tensor.matmul(
                        pview,
                        lhsT,
                        rhs,
                        start=(wi == 0),
                        stop=(wi == NW - 1),
                    )
                    if prev_mm[0] is not None:
                        bass._add_dep_helper(
                            mm.ins, prev_mm[0].ins, sync=False,
                            reason="pe-stream-order")
                    prev_mm[0] = mm

              for gi, (b0, k) in enumerate(groups):
                n = k * width
                psum = psums[gi]
                out_sb = outpool.tile([128, n], F32, tag="osb")
                pflat = psum[:, :]
                if use_prelu:
                    # single ACT op: out = prelu(psum + bias, alpha)
                    # (HW-verified exact; CoreSim lacks Prelu)
                    nc.scalar.activation(
                        out_sb[:, :], pflat,
                        mybir.ActivationFunctionType.Prelu,
                        bias=(bias_t[:, :] if has_bias else 0.0),
                        scale=1.0, alpha=alpha_t[:, :])
                else:
                    az = azpool.tile([128, n], F32, tag="az")
                    nc.vector.tensor_copy(az[:, :], pflat)
                    nc.vector.scalar_tensor_tensor(
                        out_sb[:, :], az[:, :], alpha_t[:, :], az[:, :],
                        mybir.AluOpType.mult, mybir.AluOpType.max)

                # one store per 4-row block: y[co, 4hb:4hb+4, :width]
                # <- [128=4co+j, width]; co-major partitions match dst order
                for bb in range(k):
                    hb = p * NBLK + b0 + bb
                    dst = y_d.ap()[:, 4 * hb:4 * hb + 4, 0:width]
                    store_eng[0] += 1
                    # near the end, drain stores on two queues
                    if p >= NPART - 2 and store_eng[0] % 2 == 0:
                        eng = nc.scalar
                    else:
                        eng = nc.sync
                    eng.dma_start(dst,
                                  out_sb[:, bb * width:(bb + 1) * width])

    nc.compile()
    return nc


def kernel(x, weight, bias, alpha, widths, _trace=False):
    global LAST_RESULT
    x = np.ascontiguousarray(np.asarray(x, dtype=np.float32))
    weight = np.asarray(weight, dtype=np.float32)
    bias = np.asarray(bias, dtype=np.float32)
    alpha = np.asarray(alpha, dtype=np.float32)
    widths_np = np.asarray(widths, dtype=np.int32)
    wlist = [int(v) for v in widths_np]
    assert x.shape == (B, CI, H, W)
    for wv in wlist:
        # the block-wraparound trick requires masked-zero cols at [510,512)
        assert 4 <= wv <= W - 6, f"width {wv} outside supported range"

    w1, w2 = _build_weights(weight)
    alpha_p = np.ascontiguousarray(np.repeat(alpha, 4)[:, None].astype(np.float32))
    has_bias = bool(np.any(bias != 0.0))

    nc = _build_program(wlist, has_bias)

    shared = {"w1": w1, "w2": w2, "alpha_p": alpha_p}
    if has_bias:
        shared["bias_p"] = np.ascontiguousarray(
            np.repeat(bias, 4)[:, None].astype(np.float32))
        shared["abias_p"] = np.ascontiguousarray(
            (np.repeat(bias, 4) * np.repeat(alpha, 4))[:, None].astype(np.float32))
    in_maps = [dict(shared, x=np.ascontiguousarray(x[b])) for b in range(B)]

    res = run_bass_kernel_spmd(nc, in_maps, list(range(NCORES)),
                               trace=_trace)
    LAST_RESULT = res
    y = np.stack([res.results[c]["y"] for c in range(NCORES)], axis=0)
    return y.astype(np.float32)


if __name__ == "__main__":
    # smoke test with random data (no reference comparison)
    rng = np.random.default_rng(0)
    x = rng.standard_normal((B, CI, H, W), dtype=np.float32)
    weight = (rng.standard_normal((CO, CI, 5, 5)) * 0.05).astype(np.float32)
    bias = np.zeros(CO, np.float32)
    alpha = np.full(CO, 0.25, np.float32)
    lat = (np.arange(NPART) + 0.5) / NPART * np.pi - np.pi / 2.0
    widths = np.maximum(((np.cos(lat) * W).astype(np.int32) // 2) * 2, 16)
    y = kernel(x, weight, bias, alpha, widths.astype(np.int32))
    print("out", y.shape, y.dtype, float(np.abs(y).max()))


# revision 34
# speedup vs baseline: 1.3517x; 1.1673x over previous
"""Trainium2 Bass kernel for nn_EntropyConv (masked 5x5 PixelCNN-style conv,
per-latitude-partition padding + width masking + PReLU).

Strategy: data-parallel over batch (8 cores x 1 batch element). Per core,
a row-phase-split SBUF layout puts (row mod 4, ci) on the 128 K-partitions
so each PSUM tile computes 4 output rows x 32 channels with K=M=128 fp32r
matmuls: 10 matmuls per tile (5 kw shifts x 2 row-windows), kh taps encoded
in host-precomputed block-Toeplitz weight matrices.
"""

import sys
import os
from contextlib import ExitStack

import numpy as np

sys.path.insert(0, "/opt/trn_rl_repo")

import concourse.bass as bass  # noqa: E402
import concourse.tile as tile  # noqa: E402
from concourse import bacc, mybir  # noqa: E402
from concourse import bass_utils  # noqa: E402
from concourse.bass_utils import run_bass_kernel_spmd  # noqa: E402

# Enable walrus's redundant-LDWEIGHTS elimination: our matmul stream reuses
# each stationary weight across 8 consecutive matmuls, and the default
# --enable-ldw-opt=false forces a ~190ns weight reload per matmul (~40% of
# PE time). Correctness is verified against the fp32 reference.
if not os.environ.get("BASS_NO_LDWOPT"):
    _orig_run_command = bass_utils.run_command

    def _run_command_ldwopt(argv, **kwargs):
        argv = ["--enable-ldw-opt=true" if a == "--enable-ldw-opt=false" else a
                for a in argv]
        return _orig_run_command(argv, **kwargs)

    if bass_utils.run_command is not _run_command_ldwopt:
        bass_utils.run_command = _run_command_ldwopt

# Model constants (hardcoded per problem spec)
NGROUPS, CIN, COUT, KSIZE, NPART = 8, 4, 4, 5, 8
B, H, W = 8, 256, 512
CI = NGROUPS * CIN   # 32
CO = NGROUPS * COUT  # 32
Hp = H // NPART      # 32 rows per latitude chunk
NBLK = Hp // 4       # 8 four-row blocks per chunk
NCORES = 8
F32 = mybir.dt.float32
F32R = mybir.dt.float32r

# X4 tile: 2 guard cols + 9 blocks of 512 + 2 guard cols
XBLK = 9
XLEN = 2 + XBLK * W + 2

LAST_RESULT = None  # BassKernelResults from the most recent run (for test.py)


def _group_mask():
    """PixelCNN group mask for 5x5 kernel, mask-B (hidden) variant."""
    m = np.zeros((CO, CI, KSIZE, KSIZE), np.float32)
    c = KSIZE // 2
    m[:, :, :c, :] = 1.0
    m[:, :, c, :c] = 1.0
    gin = np.arange(CI) // CIN
    gout = np.arange(CO) // COUT
    center = gin[None, :] <= gout[:, None]
    m[:, :, c, c] = center.astype(np.float32)
    return m


def _build_weights(weight):
    """Block-Toeplitz lhsT matrices.

    w1/w2[kw, 32*rp+ci, 32*j+co]: contribution of input row (4h4+rp-2)
    [w1] or (4h4+rp+2) [w2] to output row (4h4+j), i.e. kh = rp-j [w1]
    or rp-j+4 [w2], valid when 0 <= kh < 5.
    """
    wm = (weight * _group_mask()).astype(np.float32)  # [co, ci, kh, kw]
    w1 = np.zeros((KSIZE, 128, 128), np.float32)
    w2 = np.zeros((KSIZE, 128, 128), np.float32)
    for rp in range(4):
        for j in range(4):
            kh1 = rp - j
            kh2 = rp - j + 4
            # [ci, co] block at rows 32*rp+ci, cols 4*co+j (co-major
            # output partitions -> contiguous 4-row HBM stores)
            if 0 <= kh1 < KSIZE:
                for kw in range(KSIZE):
                    w1[kw, 32 * rp:32 * rp + 32, j::4] = wm[:, :, kh1, kw].T
            if 0 <= kh2 < KSIZE:
                for kw in range(KSIZE):
                    w2[kw, 32 * rp:32 * rp + 32, j::4] = wm[:, :, kh2, kw].T
    return w1, w2


def _tile_groups(width):
    """Split the 8 output blocks of a chunk into PSUM tile groups of k
    blocks, keeping k*width <= 512 (one PSUM bank) and preferring
    k*width >= 256 (fp32r full-rate threshold)."""
    if width >= 256:
        return [(b, 1) for b in range(NBLK)]
    k = min(NBLK, 512 // width)
    groups = []
    b = 0
    while b < NBLK:
        kk = min(k, NBLK - b)
        groups.append((b, kk))
        b += kk
    return groups


def _build_program(widths, has_bias, use_prelu=True):
    nc = bacc.Bacc("TRN2", target_bir_lowering=False, debug=False,
                   num_devices=NCORES)

    x_d = nc.dram_tensor("x", [CI, H, W], F32R, kind="ExternalInput")
    w1_d = nc.dram_tensor("w1", [KSIZE, 128, 128], F32R, kind="ExternalInput")
    w2_d = nc.dram_tensor("w2", [KSIZE, 128, 128], F32R, kind="ExternalInput")
    alpha_d = nc.dram_tensor("alpha_p", [128, 1], F32, kind="ExternalInput")
    if has_bias:
        bias_d = nc.dram_tensor("bias_p", [128, 1], F32, kind="ExternalInput")
    y_d = nc.dram_tensor("y", [CO, H, W], F32, kind="ExternalOutput")

    # DRAM views for phase-split access
    # x rows: global row = 4*hb + r
    x_r = x_d.ap().rearrange("ci (hb r) w -> r ci hb w", r=4)

    with tile.TileContext(nc) as tc, ExitStack() as ctx:
        wpool = ctx.enter_context(tc.tile_pool(name="wts", bufs=1))
        spool = ctx.enter_context(tc.tile_pool(name="scalars", bufs=1))
        x4pool = ctx.enter_context(tc.tile_pool(name="x4", bufs=3))
        psumpool = ctx.enter_context(
            tc.tile_pool(name="psum", bufs=8, space=bass.MemorySpace.PSUM))
        outpool = ctx.enter_context(tc.tile_pool(name="outsb", bufs=24))
        azpool = ctx.enter_context(tc.tile_pool(name="azp", bufs=6))

        wt1 = wpool.tile([128, KSIZE, 128], F32R, tag="w1")
        wt2 = wpool.tile([128, KSIZE, 128], F32R, tag="w2")
        w1v = w1_d.ap().rearrange("kw k m -> k kw m")
        w2v = w2_d.ap().rearrange("kw k m -> k kw m")
        for kw in range(KSIZE):
            nc.sync.dma_start(wt1[:, kw, :], w1v[:, kw, :])
            nc.scalar.dma_start(wt2[:, kw, :], w2v[:, kw, :])
        alpha_t = spool.tile([128, 1], F32, tag="alpha")
        nc.sync.dma_start(alpha_t[:], alpha_d.ap())
        if has_bias:
            bias_t = spool.tile([128, 1], F32, tag="bias")
            nc.sync.dma_start(bias_t[:], bias_d.ap())

        prev_mm = [None]

        store_eng = [0]
        for p in range(NPART):
            width = widths[p]
            x4 = x4pool.tile([128, XLEN], F32R, tag="x4")
            x4f = x4[:, :]

            # load the 4 rp groups (valid cols only)
            for rp in range(4):
                if rp < 2:
                    r, bdst = rp + 2, 1
                    # rows 4b+rp-2 for b=1..8 -> r=(rp+2), hb = p*8 + b-1
                    src = x_r[r][:, p * NBLK:p * NBLK + NBLK, 0:width]
                else:
                    r = rp - 2
                    # rows 4b+rp-2 for b=0..7 -> r=(rp-2), hb = p*8 + b
                    src = x_r[r][:, p * NBLK:p * NBLK + NBLK, 0:width]
                    bdst = 0
                dst = x4f[32 * rp:32 * rp + 32,
                          2 + bdst * W:2 + (bdst + NBLK) * W].rearrange(
                              "q (b x) -> q b x", x=W)[:, :, 0:width]
                nc.gpsimd.dma_start(dst, src)

            # guards (left/right 2 cols)
            nc.vector.memset(x4f[:, 0:2].bitcast(F32), 0.0)
            nc.vector.memset(x4f[:, XLEN - 2:XLEN].bitcast(F32), 0.0)
            # pad blocks: rp 0,1 -> block 0 ; rp 2,3 -> block 8
            nc.vector.memset(x4f[0:64, 2:2 + W].bitcast(F32), 0.0)
            nc.vector.memset(x4f[64:128, 2 + 8 * W:2 + 9 * W].bitcast(F32), 0.0)
            # 2-col zero strips: [width, width+2) and [510, 512) in each block
            blocks_view = x4f[:, 2:2 + XBLK * W].rearrange(
                "q (b x) -> q b x", x=W)
            nc.vector.memset(blocks_view[:, :, width:width + 2].bitcast(F32), 0.0)
            if width + 2 < W - 2:
                nc.vector.memset(blocks_view[:, :, W - 2:W].bitcast(F32), 0.0)

            all_groups = _tile_groups(width)
            halves = [all_groups[:(len(all_groups) + 1) // 2],
                      all_groups[(len(all_groups) + 1) // 2:]]

            for groups in halves:
              if not groups:
                  continue
              psums = []
              for (b0, k) in groups:
                ps_t = psumpool.tile([128, k * width], F32, tag="ps")
                psums.append(ps_t)

              # weight-major: each stationary weight is reused across all
              # groups back-to-back so walrus's ldw-opt elides the reloads
              NW = 2 * KSIZE
              for wi in range(NW):
                m, kw = divmod(wi, KSIZE)
                lhsT = (wt1 if m == 0 else wt2)[:, kw, :]
                for gi, (b0, k) in enumerate(groups):
                    s = 2 + (b0 + m) * W + (kw - 2)
                    rhs = x4f[:, s:s + k * W].rearrange(
                        "q (b x) -> q b x", x=W)[:, :, 0:width]
                    pview = psums[gi][:, :].rearrange(
                        "q (b x) -> q b x", x=width)
                    mm = nc.tensor.matmul(
                        pview,
                        lhsT,
                        rhs,
                        start=(wi == 0),
                        stop=(wi == NW - 1),
                    )
                    if prev_mm[0] is not None:
                        bass._add_dep_helper(
                            mm.ins, prev_mm[0].ins, sync=False,
                            reason="pe-stream-order")
                    prev_mm[0] = mm

              for gi, (b0, k) in enumerate(groups):
                n = k * width
                psum = psums[gi]
                out_sb = outpool.tile([128, n], F32, tag="osb")
                pflat = psum[:, :]
                if use_prelu:
                    # single ACT op: out = prelu(psum + bias, alpha)
                    # (HW-verified exact; CoreSim lacks Prelu)
                    nc.scalar.activation(
                        out_sb[:, :], pflat,
                        mybir.ActivationFunctionType.Prelu,
                        bias=(bias_t[:, :] if has_bias else 0.0),
                        scale=1.0, alpha=alpha_t[:, :])
                else:
                    az = azpool.tile([128, n], F32, tag="az")
                    nc.vector.tensor_copy(az[:, :], pflat)
                    nc.vector.scalar_tensor_tensor(
                        out_sb[:, :], az[:, :], alpha_t[:, :], az[:, :],
                        mybir.AluOpType.mult, mybir.AluOpType.max)

                # one store per 4-row block: y[co, 4hb:4hb+4, :width]
                # <- [128=4co+j, width]; co-major partitions match dst order
                for bb in range(k):
                    hb = p * NBLK + b0 + bb
                    dst = y_d.ap()[:, 4 * hb:4 * hb + 4, 0:width]
                    store_eng[0] += 1
                    # near the end, drain stores on two queues
                    if p >= NPART - 2 and store_eng[0] % 2 == 0:
                        eng = nc.scalar
                    else:
                        eng = nc.sync
                    eng.dma_start(dst,
                                  out_sb[:, bb * width:(bb + 1) * width])

    nc.compile()
    return nc


def kernel(x, weight, bias, alpha, widths, _trace=False):
    global LAST_RESULT
    x = np.ascontiguousarray(np.asarray(x, dtype=np.float32))
    weight = np.asarray(weight, dtype=np.float32)
    bias = np.asarray(bias, dtype=np.float32)
    alpha = np.asarray(alpha, dtype=np.float32)
    widths_np = np.asarray(widths, dtype=np.int32)
    wlist = [int(v) for v in widths_np]
    assert x.shape == (B, CI, H, W)
    for wv in wlist:
        # the block-wraparound trick requires masked-zero cols at [510,512)
        assert 4 <= wv <= W - 6, f"width {wv} outside supported range"

    w1, w2 = _build_weights(weight)
    alpha_p = np.ascontiguousarray(np.repeat(alpha, 4)[:, None].astype(np.float32))
    has_bias = bool(np.any(bias != 0.0))

    nc = _build_program(wlist, has_bias)

    shared = {"w1": w1, "w2": w2, "alpha_p": alpha_p}
    if has_bias:
        shared["bias_p"] = np.ascontiguousarray(
            np.repeat(bias, 4)[:, None].astype(np.float32))
    in_maps = [dict(shared, x=np.ascontiguousarray(x[b])) for b in range(B)]

    res = run_bass_kernel_spmd(nc, in_maps, list(range(NCORES)),
                               trace=_trace)
    LAST_RESULT = res
    y = np.stack([res.results[c]["y"] for c in range(NCORES)], axis=0)
    return y.astype(np.float32)


if __name__ == "__main__":
    # smoke test with random data (no reference comparison)
    rng = np.random.default_rng(0)
    x = rng.standard_normal((B, CI, H, W), dtype=np.float32)
    weight = (rng.standard_normal((CO, CI, 5, 5)) * 0.05).astype(np.float32)
    bias = np.zeros(CO, np.float32)
    alpha = np.full(CO, 0.25, np.float32)
    lat = (np.arange(NPART) + 0.5) / NPART * np.pi - np.pi / 2.0
    widths = np.maximum(((np.cos(lat) * W).astype(np.int32) // 2) * 2, 16)
    y = kernel(x, weight, bias, alpha, widths.astype(np.int32))
    print("out", y.shape, y.dtype, float(np.abs(y).max()))
